# revision 52
# baseline (speedup 1.0000x reference)
"""Trainium2 Bass kernel for nn_Block_4526895530469 (Mamba block + MLP residual).

Sharding over 8 NeuronCores: core c -> batch b=c//4, channel shard r=c%4
(512 of the 2048 d_inner channels), full T=2048 sequence per core. The
selective scan runs full-T per channel on the Vector engine
(tensor_tensor_scan), so there is no cross-core state chain. Layout for the
scan is (s,e)-pairs on partitions (row p = 8*s + e_local, 16 states x 8
channels per 128-row tile) x time on the free dimension. All P6 elementwise
work stays on DVE (the Pool engine is ~3x slower per op and its in-order
queue couples it into the scan's critical chain); bb/t1 are batched over
j-pairs and the D*u skip term is folded into the y PSUM accumulation as a
diagonal matmul.

Collectives (all within each batch group of 4 cores): one AllReduce for the
(96, T) x_dbl partial sums (contraction over the sharded d_inner), and one
2-half ReduceScatter for the out_proj partials which simultaneously scatters
tokens for the token-parallel MLP tail.

All inputs arrive in ONE packed bf16 dram tensor (fp8/f32 sections via
AP.bitcast): per-exec dispatch costs ~34us PER INPUT BUFFER on this stack,
so buffer count - not bytes - sets the measurement floor. Wire dtypes are
bf16 (x, out) with the residual path reconstructed in f32 on device.
"""
import sys
sys.path.insert(0, '/opt/trn_rl_repo')

import numpy as np
from contextlib import ExitStack

import concourse.bass as bass
from concourse import bacc
import concourse.tile as tile
from concourse import mybir
from concourse.bass_utils import run_bass_kernel_spmd

# The interp (used by Tile's scheduling pass and by test simulation) lacks
# Silu; emulate it: run the existing Sigmoid path, then multiply by the
# scaled/biased input.
from concourse import bass_interp as _bi
from concourse import mybir as _mb

_orig_visit_act = _bi.InstructionExecutor.visit_InstActivation


def _visit_act_with_silu(self, instruction, *a, **kw):
    if instruction.func != _mb.ActivationFunctionType.Silu:
        return _orig_visit_act(self, instruction, *a, **kw)
    import numpy as _np
    assert len(instruction.outs) == 1, "Silu shim: no accum_out support"
    func0 = instruction.func
    try:
        instruction.func = _mb.ActivationFunctionType.Sigmoid
        res = _orig_visit_act(self, instruction, *a, **kw)
    finally:
        instruction.func = func0
    reg_snapshot = kw.get("reg_snapshot")
    inp = self.view_ap(instruction.ins[0], _bi.Direction.READ, instruction,
                       reg_snapshot=reg_snapshot).astype(_np.float32)
    inp = inp.reshape(inp.shape[0], -1)

    def _val(arg):
        if isinstance(arg, _mb.ImmediateValue):
            return arg.value
        v = self.view_ap(arg, _bi.Direction.READ, instruction,
                         reg_snapshot=reg_snapshot).astype(_np.float32)
        return v.reshape(v.shape[0], -1)

    bias = _val(instruction.ins[1])
    scale = _val(instruction.ins[2])
    sx = inp * scale + bias
    out_view = self.view_ap(instruction.outs[0], _bi.Direction.WRITE, instruction,
                            reg_snapshot=reg_snapshot)
    sig = _np.asarray(out_view, dtype=_np.float32).reshape(sx.shape)
    out_view[:] = (sig * sx).reshape(out_view.shape).astype(out_view.dtype)
    return res


_bi.InstructionExecutor.visit_InstActivation = _visit_act_with_silu

F32 = mybir.dt.float32
BF16 = mybir.dt.bfloat16
FP8 = mybir.dt.float8e4
AF = mybir.ActivationFunctionType
ALU = mybir.AluOpType

D_MODEL, D_INNER, D_STATE, D_CONV, DT_RANK = 1024, 2048, 16, 4, 64
B, T = 2, 2048
EL = D_INNER // 4          # 512 channels per core
NET = EL // 128            # 4 e-tiles
NJ = EL // 8               # 64 scan tiles
NCH = T // 512             # 4 t-chunks
TQ = T // 4                # 512 tokens for the MLP tail
XD = DT_RANK + 2 * D_STATE  # 96
EPS = float(np.finfo(np.float32).eps)

_CACHE = {}


class _SkipBlock(Exception):
    pass


from contextlib import contextmanager


@contextmanager
def _skippable():
    try:
        yield
    except _SkipBlock:
        pass


def _build(nocc=False, gps_mod=0, ar_bf16=True, nocc_ar=False, nocc_rs=False, rs_split=True,
           ab_no_p6=False, ab_scan_tt=False, ab_no_dtur=False, ab_no_p1p2=False,
           ab_no_mlp=False, bb_pool_mod=0):
    nc = bacc.Bacc("TRN2", target_bir_lowering=False, debug=False, num_devices=8)

    def din(name, shape, dt=BF16):
        return nc.dram_tensor(name, list(shape), dt, kind="ExternalInput").ap()

    # ALL inputs are packed into ONE bf16-typed buffer: the per-exec dispatch
    # cost is ~34us PER INPUT BUFFER, so buffer count dominates the measured
    # floor. fp8/f32 sections are bitcast views of bf16 rows. Row layout is
    # mirrored in _prep_inputs.
    #   rows 0:6760     native bf16 tensors (1024-wide, row-major flattened)
    #   rows 6760:7528  fp8 section   (bitcast -> [768, 2048] fp8)
    #   rows 7528:7656  f32 section   (bitcast -> [128, 512] f32)
    pball = din("pball", (7656, 1024))
    pbf = pball
    pf8 = pball[6760:7528, :].bitcast(FP8)     # [768, 2048] fp8
    pf32 = pball[7528:7656, :].bitcast(F32)    # [128, 512] f32

    # offsets: fp8 rows are in the bitcast [768, 2048] view (w_in row k ->
    # view rows [64k:64(k+1)] hold 128x1024 fp8)
    PBF_XB, PBF_XQ, PBF_WFC, PBF_WPR = 0, 2048, 2560, 4608
    PBF_WXP, PBF_WDT, PBF_IDB = 6656, 6704, 6736
    PBF_S01N, PBF_S01P, PBF_R8, PBF_G8, PBF_ONES = 6752, 6754, 6756, 6757, 6758
    PF32_CWC, PF32_CB, PF32_DTB, PF32_NEGA, PF32_DC, PF32_IDF = 0, 16, 20, 24, 88, 92

    out = nc.dram_tensor("out", [TQ, D_MODEL], BF16, kind="ExternalOutput").ap()

    xdbl_in_h = [nc.dram_tensor(f"xdbl_in{h}", [XD, T // 2], BF16).ap() for h in range(2)]
    xdbl_out_h = [nc.dram_tensor(f"xdbl_out{h}", [XD, T // 2], BF16).ap() for h in range(2)]
    rs_in_h = [nc.dram_tensor(f"rs_in{h}", [4 * D_MODEL, TQ // 2], BF16).ap() for h in range(2)]
    rs_out_h = [nc.dram_tensor(f"rs_out{h}", [D_MODEL, TQ // 2], BF16).ap() for h in range(2)]
    dtu_dram = nc.dram_tensor("dtu_dram", [EL, T], BF16).ap()
    xqT_dram = nc.dram_tensor("xqT_dram", [D_MODEL, TQ], BF16).ap()

    g8 = [[0, 1, 2, 3, 4, 5, 6, 7]]
    g4 = [[0, 1, 2, 3], [4, 5, 6, 7]]

    with tile.TileContext(nc) as tc, ExitStack() as top:
        cpool = top.enter_context(tc.tile_pool(name="consts", bufs=1))

        def cload(nm, name_ap, shape, dt=BF16):
            t = cpool.tile(list(shape), dt, tag=nm, name=nm)
            nc.sync.dma_start(t[:], name_ap)
            return t

        negA_t = cload("negA_t", pf32[:, PF32_NEGA:PF32_NEGA + NJ], (128, NJ), F32)
        cwc_t = cload("cwc_t", pf32[:, PF32_CWC:PF32_CWC + NET * D_CONV],
                      (128, NET * D_CONV), F32)
        convb_t = cload("convb_t", pf32[:, PF32_CB:PF32_CB + NET], (128, NET), F32)
        dtbn_t = cload("dtbn_t", pf32[:, PF32_DTB:PF32_DTB + NET], (128, NET), F32)
        dcols_t = cload("dcols_t", pf32[:, PF32_DC:PF32_DC + NET], (128, NET), F32)
        s01n_t = cload("s01n_t", pbf[PBF_S01N:PBF_S01N + 2, :], (D_STATE, 128))
        s01p_t = cload("s01p_t", pbf[PBF_S01P:PBF_S01P + 2, :], (D_STATE, 128))
        idb_t = cload("idb_t", pbf[PBF_IDB:PBF_IDB + 16, :], (128, 128))
        idf_t = cload("idf_t", pf32[:, PF32_IDF:PF32_IDF + 128], (128, 128), F32)
        ones_t = cload("ones_t", pbf[PBF_ONES:PBF_ONES + 1, 0:128], (128, 1))
        onesr_t = cload("onesr_t", pbf[PBF_ONES:PBF_ONES + 1, 0:128], (1, 128))
        eps_t = cpool.tile([128, 1], F32)
        nc.vector.memset(eps_t[:], EPS)
        eps256_t = cpool.tile([128, 1], F32)
        nc.vector.memset(eps256_t[:], EPS / 256.0)
        c16_t = cpool.tile([128, 1], F32)
        nc.vector.memset(c16_t[:], 16.0)
        cinv4k_t = cpool.tile([128, 1], F32)
        nc.vector.memset(cinv4k_t[:], 1.0 / 4096.0)
        # conv taps / D as [128,128] diag blocks: ident * per-partition column
        convd_t = cpool.tile([128, NET * D_CONV * 128], BF16, tag="convd_t", name="convd_t")
        for b in range(NET * D_CONV):
            eng = nc.vector if b % 2 else nc.gpsimd
            eng.tensor_scalar_mul(convd_t[:, 128 * b:128 * (b + 1)], idb_t[:],
                                  cwc_t[:, b:b + 1])
        ddiag_t = cpool.tile([128, NET * 128], BF16, tag="ddiag_t", name="ddiag_t")
        for b in range(NET):
            eng = nc.vector if b % 2 else nc.gpsimd
            eng.tensor_scalar_mul(ddiag_t[:, 128 * b:128 * (b + 1)], idb_t[:],
                                  dcols_t[:, b:b + 1])
        # r01/g01 selection tables from tiny row/col patterns
        r01_t = cpool.tile([128, 16 * 128], BF16)
        nc.vector.memset(r01_t[:], 0.0)
        for jm in range(16):
            nc.sync.dma_start(r01_t[8 * jm:8 * jm + 8, 128 * jm:128 * (jm + 1)],
                              pbf[PBF_R8:PBF_R8 + 1, :])
        g01_t = cpool.tile([128, 16 * 128], BF16)
        nc.gpsimd.memset(g01_t[:], 0.0)
        for jm in range(16):
            nc.sync.dma_start(g01_t[:, 136 * jm:136 * jm + 8], pbf[PBF_G8:PBF_G8 + 1, :])

        # long-lived activations through the scan phase (freed before P7 so
        # the MLP weights can prefetch under out_proj + ReduceScatter)
        y2p = top.enter_context(tc.tile_pool(name="y2p", bufs=1))
        y2f8 = y2p.tile([128, NET, T], FP8, tag="y2f8", name="y2f8")
        actsx = top.enter_context(ExitStack())
        acts = actsx.enter_context(tc.tile_pool(name="acts", bufs=1))
        u_bf = [acts.tile([128, T], BF16, tag=f"u{k}", name=f"u{k}") for k in range(NET)]
        sz_bf = [acts.tile([128, T], BF16, tag=f"sz{k}", name=f"sz{k}") for k in range(NET)]
        lnsig_bf = [acts.tile([128, T], BF16, tag=f"lns{k}", name=f"lns{k}") for k in range(NET)]
        brep_t = acts.tile([128, T], BF16, tag="brep", name="brep")
        crep_t = acts.tile([128, T], BF16, tag="crep", name="crep")
        dtlow_bf = acts.tile([DT_RANK, T], BF16, tag="dtlow", name="dtlow")

        # ============ P1+P2: rmsnorm, transpose, in_proj ============
        with ExitStack() as ph:
            winp = ph.enter_context(tc.tile_pool(name="win", bufs=1))
            # fp8 DoubleRow layout: k-tiles on a free axis so one AP can
            # address a (2k, 2k+1) pair for the double-pumped matmul
            w_in_t = winp.tile([128, 8, 2 * EL], FP8, tag="wi8", name="wi8")
            for k in range(8):
                nc.sync.dma_start(w_in_t[:, k, :], pf8[64 * k:64 * (k + 1), :])

            xinp = ph.enter_context(tc.tile_pool(name="xinz", bufs=1))
            # 3 zero pad columns in front so the causal-conv PE matmuls can
            # read shifted windows without bounds checks
            x_in = [xinp.tile([128, 3 + T], BF16, tag=f"xin{k}", name=f"xin{k}")
                    for k in range(NET)]
            for k in range(NET):
                nc.vector.memset(x_in[k][:, 0:3], 0.0)

            if ab_no_p1p2:
                for k in range(NET):
                    nc.vector.memset(x_in[k][:], 0.01)
                    nc.vector.memset(sz_bf[k][:], 0.01)
            with _skippable(), ExitStack() as p1:
                if ab_no_p1p2:
                    raise _SkipBlock
                xnp = p1.enter_context(tc.tile_pool(name="xnT", bufs=1))
                xn8 = xnp.tile([128, 8, T], FP8, tag="xn8", name="xn8")
                str_p = p1.enter_context(tc.tile_pool(name="p1s", bufs=3))
                xtmp = p1.enter_context(tc.tile_pool(name="p1x", bufs=6))
                jp = p1.enter_context(tc.tile_pool(name="p1j", bufs=1))
                junk = jp.tile([128, D_MODEL], BF16)
                pps1 = p1.enter_context(tc.tile_pool(name="p1ps", bufs=2, space="PSUM"))

                for c in range(NCH):
                    xnt4 = []
                    for q in range(4):
                        i = 4 * c + q
                        xt = str_p.tile([128, D_MODEL], BF16, tag="xt", name="xt")
                        nc.gpsimd.dma_start(xt[:], pbf[PBF_XB + 128 * i:PBF_XB + 128 * (i + 1), :])
                        ssq = str_p.tile([128, 1], F32, tag="ssq", name="ssq")
                        nc.scalar.activation(junk[:], xt[:], AF.Square, accum_out=ssq[:])
                        rr = str_p.tile([128, 1], F32, tag="rr", name="rr")
                        nc.scalar.activation(rr[:], ssq[:], AF.Sqrt, scale=1.0 / D_MODEL,
                                             bias=eps_t[:, 0:1])
                        rc = str_p.tile([128, 1], F32, tag="rc", name="rc")
                        nc.vector.reciprocal(rc[:], rr[:])
                        xnt = xtmp.tile([128, D_MODEL], BF16, tag="xnt", name="xnt")
                        nc.vector.tensor_scalar_mul(xnt[:], xt[:], rc[:, 0:1])
                        xnt4.append(xnt)
                    for k in range(8):
                        pt = pps1.tile([128, 512], BF16, tag="pt", name="pt")
                        for q in range(4):
                            nc.tensor.transpose(pt[:, 128 * q:128 * (q + 1)],
                                                xnt4[q][:, 128 * k:128 * (k + 1)],
                                                idb_t[:])
                        # cast to fp8 with a x16 range shift (fp8e4 min-normal
                        # is 2^-6; unscaled unit-RMS values lose mantissa);
                        # alternate engines to balance Act vs DVE
                        if (k + c) % 2:
                            nc.vector.tensor_scalar_mul(xn8[:, k, 512 * c:512 * (c + 1)],
                                                        pt[:], c16_t[:, 0:1])
                        else:
                            nc.scalar.activation(xn8[:, k, 512 * c:512 * (c + 1)], pt[:],
                                                 AF.Copy, scale=16.0)

                # in_proj: fp8 DoubleRow (two 128-deep k-tiles per matmul)
                pps2 = p1.enter_context(tc.tile_pool(name="p2ps", bufs=4, space="PSUM"))
                UNSC = 1.0 / (256.0 * 16.0)
                for m in range(8):
                    for c in range(NCH):
                        ps = pps2.tile([128, 512], F32, tag="ps", name="ps")
                        for p2 in range(4):
                            nc.tensor.matmul(ps[:], w_in_t[:, 2 * p2:2 * p2 + 2,
                                                          128 * m:128 * (m + 1)],
                                             xn8[:, 2 * p2:2 * p2 + 2,
                                                 512 * c:512 * (c + 1)],
                                             start=(p2 == 0), stop=(p2 == 3),
                                             perf_mode=mybir.MatmulPerfMode.DoubleRow)
                        if m < 4:
                            nc.vector.tensor_scalar_mul(x_in[m][:, 3 + 512 * c:3 + 512 * (c + 1)],
                                                        ps[:], cinv4k_t[:, 0:1])
                        else:
                            nc.scalar.activation(sz_bf[m - 4][:, 512 * c:512 * (c + 1)],
                                                 ps[:], AF.Silu, scale=UNSC)

            # ============ P3: conv (PE diag-matmuls) + silu -> u ============
            with ExitStack() as p3:
                pps3 = p3.enter_context(tc.tile_pool(name="p3ps", bufs=2, space="PSUM"))
                for k in range(NET):
                    for c in range(NCH):
                        ps = pps3.tile([128, 512], F32, tag="psc", name="psc")
                        for tap in range(D_CONV):
                            sh = D_CONV - 1 - tap       # time shift for this tap
                            dcol = 128 * (D_CONV * k + tap)
                            nc.tensor.matmul(ps[:], convd_t[:, dcol:dcol + 128],
                                             x_in[k][:, 3 + 512 * c - sh:3 + 512 * (c + 1) - sh],
                                             start=(tap == 0), stop=(tap == D_CONV - 1))
                        nc.scalar.activation(u_bf[k][:, 512 * c:512 * (c + 1)], ps[:],
                                             AF.Silu, bias=convb_t[:, k:k + 1])

        # ============ P4: x_proj partial -> AllReduce8 -> dtlow/B_rep/C_rep ============
        with ExitStack() as p4:
            wxp = p4.enter_context(tc.tile_pool(name="wxp", bufs=1))
            w_xp_t = [wxp.tile([128, XD], BF16, tag=f"wxp{k}", name=f"wxp{k}") for k in range(NET)]
            for k in range(NET):
                nc.sync.dma_start(w_xp_t[k][:], pbf[PBF_WXP + 12 * k:PBF_WXP + 12 * (k + 1), :])
            pps = p4.enter_context(tc.tile_pool(name="p4ps", bufs=2, space="PSUM"))
            sp = p4.enter_context(tc.tile_pool(name="p4s", bufs=2))
            big = p4.enter_context(tc.tile_pool(name="p4big", bufs=1))
            # split the AllReduce into t-halves: AR(half 0) runs while the
            # half-1 x_proj partials are still being computed, and the
            # dt/B/C consumers of half 0 can start under AR(half 1)
            for half in range(2):
                for c in (2 * half, 2 * half + 1):
                    ps = pps.tile([XD, 512], F32, tag="ps4", name="ps4")
                    for k in range(NET):
                        nc.tensor.matmul(ps[:], w_xp_t[k][:], u_bf[k][:, 512 * c:512 * (c + 1)],
                                         start=(k == 0), stop=(k == NET - 1))
                    t0 = sp.tile([XD, 512], BF16, tag="t0", name="t0")
                    if c % 2:
                        nc.vector.tensor_copy(t0[:], ps[:])
                    else:
                        nc.scalar.copy(t0[:], ps[:])
                    nc.sync.dma_start(xdbl_in_h[half][:, 512 * (c % 2):512 * (c % 2 + 1)],
                                      t0[:])
                if nocc or nocc_ar:
                    nc.sync.dma_start(xdbl_out_h[half], xdbl_in_h[half])
                else:
                    # per-batch-group AllReduce: cores {0-3} hold batch 0's
                    # channel shards, {4-7} batch 1's
                    nc.gpsimd.collective_compute("AllReduce", ALU.add, replica_groups=g4,
                                                 ins=[xdbl_in_h[half]],
                                                 outs=[xdbl_out_h[half]])
            # fill the AllReduce window: transpose the residual token-quarter
            # now and stage it in DRAM for the MLP tail
            ppq = p4.enter_context(tc.tile_pool(name="p4q", bufs=2, space="PSUM"))
            spq = p4.enter_context(tc.tile_pool(name="p4qs", bufs=2))
            for i in range(TQ // 128):
                xt_ = spq.tile([128, D_MODEL], BF16, tag="xq_tm", name="xq_tm")
                nc.sync.dma_start(xt_[:], pbf[PBF_XQ + 128 * i:PBF_XQ + 128 * (i + 1), :])
                for h in range(2):
                    ptq = ppq.tile([128, 512], BF16, tag="ptq", name="ptq")
                    for q in range(4):
                        k = 4 * h + q
                        nc.tensor.transpose(ptq[:, 128 * q:128 * (q + 1)],
                                            xt_[:, 128 * k:128 * (k + 1)], idb_t[:])
                    otq = spq.tile([128, 512], BF16, tag="otq", name="otq")
                    nc.scalar.copy(otq[:], ptq[:])
                    for q in range(4):
                        k = 4 * h + q
                        nc.sync.dma_start(
                            xqT_dram[128 * k:128 * (k + 1), 128 * i:128 * (i + 1)],
                            otq[:, 128 * q:128 * (q + 1)])
            # DMA-select the three row sections per t-half (so half 0's
            # consumers start under the half-1 AllReduce; DMA has no
            # 32-alignment restriction on source partitions)
            b_sb = big.tile([D_STATE, T], BF16, tag="b_sb", name="b_sb")
            c_sb = big.tile([D_STATE, T], BF16, tag="c_sb", name="c_sb")
            for half in range(2):
                hs = slice(1024 * half, 1024 * (half + 1))
                xo = xdbl_out_h[half]
                nc.sync.dma_start(dtlow_bf[:, hs], xo[0:DT_RANK, :])
                nc.sync.dma_start(b_sb[:, hs], xo[DT_RANK:DT_RANK + D_STATE, :])
                nc.sync.dma_start(c_sb[:, hs], xo[DT_RANK + D_STATE:XD, :])
            pps2 = p4.enter_context(tc.tile_pool(name="p4ps2", bufs=2, space="PSUM"))
            for c in range(NCH):
                pb = pps2.tile([128, 512], F32, tag="pb", name="pb")
                nc.tensor.matmul(pb[:], s01n_t[:], b_sb[:, 512 * c:512 * (c + 1)],
                                 start=True, stop=True)
                nc.vector.tensor_copy(brep_t[:, 512 * c:512 * (c + 1)], pb[:])
                pc = pps2.tile([128, 512], F32, tag="pc", name="pc")
                nc.tensor.matmul(pc[:], s01p_t[:], c_sb[:, 512 * c:512 * (c + 1)],
                                 start=True, stop=True)
                nc.vector.tensor_copy(crep_t[:, 512 * c:512 * (c + 1)], pc[:])

        # ============ P5: dt path ============
        with ExitStack() as p5:
            wdt = p5.enter_context(tc.tile_pool(name="wdt", bufs=1))
            w_dt_t = wdt.tile([DT_RANK, EL], BF16)
            nc.sync.dma_start(w_dt_t[:], pbf[PBF_WDT:PBF_WDT + 32, :])
            pps = p5.enter_context(tc.tile_pool(name="p5ps", bufs=4, space="PSUM"))
            sp = p5.enter_context(tc.tile_pool(name="p5s", bufs=2))
            sgp = p5.enter_context(tc.tile_pool(name="p5sg", bufs=1))
            # batch all Sigmoid ops, then all Ln ops: Sigmoid and Ln live in
            # different activation-function tables (1.28us reload each)
            sg_all = [sgp.tile([128, T], F32, tag=f"sg{m}", name=f"sg{m}")
                      for m in range(NET)]
            for m in range(NET):
                for c in range(NCH):
                    ps = pps.tile([128, 512], F32, tag="ps5", name="ps5")
                    nc.tensor.matmul(ps[:], w_dt_t[:, 128 * m:128 * (m + 1)],
                                     dtlow_bf[:, 512 * c:512 * (c + 1)], start=True, stop=True)
                    nc.scalar.activation(sg_all[m][:, 512 * c:512 * (c + 1)], ps[:],
                                         AF.Sigmoid, scale=-1.0, bias=dtbn_t[:, m:m + 1])
            for m in range(NET):
                nc.scalar.activation(lnsig_bf[m][:], sg_all[m][:], AF.Ln)
                dtu = sp.tile([128, T], BF16, tag="dtu", name="dtu")
                nc.vector.tensor_tensor(dtu[:], lnsig_bf[m][:], u_bf[m][:], ALU.mult)
                nc.sync.dma_start(dtu_dram[128 * m:128 * (m + 1), :], dtu[:])

        # ============ P6: scan + y-sum + gate ============
        if ab_no_p6:
            nc.vector.memset(y2f8[:], 0.01)
        with _skippable(), ExitStack() as p6:
            if ab_no_p6:
                raise _SkipBlock
            reps = p6.enter_context(tc.tile_pool(name="reps", bufs=2, space="PSUM"))
            yps = p6.enter_context(tc.tile_pool(name="ypsum", bufs=1, space="PSUM"))
            sp = p6.enter_context(tc.tile_pool(name="p6s", bufs=2))
            spa = p6.enter_context(tc.tile_pool(name="p6sa", bufs=3))
            # brep/crep duplicated side by side so bb/t1 batch over j-pairs
            # (halves the DVE instruction count for those passes)
            dupp = p6.enter_context(tc.tile_pool(name="p6dup", bufs=1))
            brep2 = dupp.tile([128, 2 * T], BF16, tag="brep2", name="brep2")
            nc.vector.tensor_copy(brep2[:, 0:T], brep_t[:])
            nc.vector.tensor_copy(brep2[:, T:2 * T], brep_t[:])
            crep2 = dupp.tile([128, 2 * T], BF16, tag="crep2", name="crep2")
            nc.vector.tensor_copy(crep2[:, 0:T], crep_t[:])
            nc.vector.tensor_copy(crep2[:, T:2 * T], crep_t[:])
            for J in range(4):
                py = yps.tile([128, T], F32, tag="py", name="py")
                # seed py with the skip term 8*D[e]*u[e,t] (diag matmul) so the
                # y2 gate reads a single finished PSUM tile
                for c in range(NCH):
                    nc.tensor.matmul(py[:, 512 * c:512 * (c + 1)],
                                     ddiag_t[:, 128 * J:128 * (J + 1)],
                                     u_bf[J][:, 512 * c:512 * (c + 1)],
                                     start=True, stop=False)
                for jp in range(8):
                    dtur2 = sp.tile([128, 2 * T], BF16, tag="dtur2", name="dtur2")
                    bb2 = sp.tile([128, 2 * T], BF16, tag="bb2", name="bb2")
                    hh2 = sp.tile([128, 2 * T], BF16, tag="hh2", name="hh2")
                    t12 = sp.tile([128, 2 * T], BF16, tag="t12", name="t12")
                    for u in range(2):
                        jj = 2 * jp + u
                        j = 16 * J + jj
                        if ab_no_dtur:
                            nc.vector.tensor_copy(dtur2[:, u * T:(u + 1) * T], u_bf[J][:])
                        else:
                            src = dtu_dram[128 * J + 8 * jj:128 * J + 8 * jj + 8, :]
                            nc.sync.dma_start(dtur2[:, u * T:(u + 1) * T],
                                              src.unsqueeze(0).broadcast_to([16, 8, T]))
                    nc.vector.tensor_tensor(bb2[:], dtur2[:], brep2[:], ALU.mult)
                    for u in range(2):
                        jj = 2 * jp + u
                        j = 16 * J + jj
                        dA = spa.tile([128, T], F32, tag="dA", name="dA")
                        for hf in range(2):
                            pr = reps.tile([128, 1024], F32, tag="pr", name="pr")
                            for q in range(2):
                                c = 2 * hf + q
                                nc.tensor.matmul(pr[:, 512 * q:512 * (q + 1)],
                                                 r01_t[:, 128 * jj:128 * (jj + 1)],
                                                 lnsig_bf[J][:, 512 * c:512 * (c + 1)],
                                                 start=True, stop=True)
                            nc.scalar.activation(dA[:, 1024 * hf:1024 * (hf + 1)], pr[:],
                                                 AF.Exp, scale=negA_t[:, j:j + 1])
                        # scan is DVE-only (walrus rejects it on Pool)
                        if ab_scan_tt:
                            nc.vector.tensor_tensor(hh2[:, u * T:(u + 1) * T], dA[:],
                                                    bb2[:, u * T:(u + 1) * T], ALU.mult)
                        else:
                            nc.vector.tensor_tensor_scan(hh2[:, u * T:(u + 1) * T], dA[:],
                                                         bb2[:, u * T:(u + 1) * T], 0.0,
                                                         ALU.mult, ALU.add)
                    nc.vector.tensor_tensor(t12[:], hh2[:], crep2[:], ALU.mult)
                    for u in range(2):
                        jj = 2 * jp + u
                        for c in range(NCH):
                            nc.tensor.matmul(py[:, 512 * c:512 * (c + 1)],
                                             g01_t[:, 128 * jj:128 * (jj + 1)],
                                             t12[:, u * T + 512 * c:u * T + 512 * (c + 1)],
                                             start=False, stop=(jj == 15))
                nc.vector.tensor_tensor(y2f8[:, J, :], py[:],
                                        sz_bf[J][:], ALU.mult)

        # free the scan-phase activations, then prefetch the MLP weights so
        # their 8MB of DMA hides under out_proj compute + the ReduceScatter
        actsx.close()
        wmlp = top.enter_context(tc.tile_pool(name="wmlp", bufs=1))
        w_fc_t = [wmlp.tile([128, 2 * D_MODEL], BF16, tag=f"wf{k}", name=f"wf{k}")
                  for k in range(8)]
        for k in range(8):
            nc.sync.dma_start(w_fc_t[k][:], pbf[PBF_WFC + 256 * k:PBF_WFC + 256 * (k + 1), :])
        w_pr_t = [wmlp.tile([128, D_MODEL], BF16, tag=f"wp{k}", name=f"wp{k}")
                  for k in range(16)]
        for k in range(16):
            nc.sync.dma_start(w_pr_t[k][:], pbf[PBF_WPR + 128 * k:PBF_WPR + 128 * (k + 1), :])

        # ============ P7: out_proj partial -> ReduceScatter4 ============
        with ExitStack() as p7:
            wout = p7.enter_context(tc.tile_pool(name="wout", bufs=1))
            w_out_t = wout.tile([128, NET, D_MODEL], FP8, tag="wo8", name="wo8")
            for k in range(NET):
                nc.sync.dma_start(w_out_t[:, k, :], pf8[512 + 64 * k:512 + 64 * (k + 1), :])
            unso_t = wout.tile([128, 1], F32, tag="unso", name="unso")
            nc.vector.memset(unso_t[:], 1.0 / 2048.0)
            pps = p7.enter_context(tc.tile_pool(name="p7ps", bufs=4, space="PSUM"))
            sp = p7.enter_context(tc.tile_pool(name="p7s", bufs=4))
            for m in range(8):
                for c in range(NCH):
                    ps = pps.tile([128, 512], F32, tag="ps7", name="ps7")
                    for p2 in range(NET // 2):
                        nc.tensor.matmul(ps[:], w_out_t[:, 2 * p2:2 * p2 + 2,
                                                        128 * m:128 * (m + 1)],
                                         y2f8[:, 2 * p2:2 * p2 + 2,
                                              512 * c:512 * (c + 1)],
                                         start=(p2 == 0), stop=(p2 == 1),
                                         perf_mode=mybir.MatmulPerfMode.DoubleRow)
                    ob = sp.tile([128, 512], BF16, tag="ob", name="ob")
                    # split the PSUM drain between Act and DVE (DVE is idle
                    # after the scan phase); 1/2048 undoes the fp8 w x256 and
                    # y2 x8 range shifts
                    if (m + c) % 2:
                        nc.vector.tensor_scalar_mul(ob[:], ps[:], unso_t[:, 0:1])
                    else:
                        nc.scalar.activation(ob[:], ps[:], AF.Copy, scale=1.0 / 2048.0)
                    rr = slice(D_MODEL * c + 128 * m, D_MODEL * c + 128 * (m + 1))
                    nc.sync.dma_start(rs_in_h[0][rr, :], ob[:, 0:TQ // 2])
                    nc.sync.dma_start(rs_in_h[1][rr, :], ob[:, TQ // 2:TQ])
            for h in range(2):
                if nocc or nocc_rs:
                    nc.sync.dma_start(rs_out_h[h], rs_in_h[h][0:D_MODEL, :])
                else:
                    nc.gpsimd.collective_compute("ReduceScatter", ALU.add,
                                                 replica_groups=g4,
                                                 ins=[rs_in_h[h]], outs=[rs_out_h[h]])

        # ============ P8: MLP tail ============
        if ab_no_mlp:
            with tc.tile_pool(name="abz", bufs=1) as abz:
                zt = abz.tile([128, D_MODEL], BF16)
                nc.vector.memset(zt[:], 0.0)
                for i in range(TQ // 128):
                    nc.sync.dma_start(out[128 * i:128 * (i + 1), :], zt[:])
        with _skippable(), ExitStack() as p8:
            if ab_no_mlp:
                raise _SkipBlock
            ar = p8.enter_context(tc.tile_pool(name="p8a", bufs=1))
            st = p8.enter_context(tc.tile_pool(name="p8t", bufs=2))
            ppt = p8.enter_context(tc.tile_pool(name="p8pt", bufs=2, space="PSUM"))
            ppm = p8.enter_context(tc.tile_pool(name="p8pm", bufs=2, space="PSUM"))
            pp1 = p8.enter_context(tc.tile_pool(name="p8p1", bufs=1, space="PSUM"))

            TH = TQ // 2
            for th in range(2):
                t0 = TH * th
                x2_T = [ar.tile([128, TH], F32, tag=f"x2T{k}", name=f"x2T{k}")
                        for k in range(8)]
                for k in range(8):
                    nc.gpsimd.dma_start(x2_T[k][:], xqT_dram[128 * k:128 * (k + 1), t0:t0 + TH])
                rsb = [ar.tile([128, TH], BF16, tag=f"rsb{k}", name=f"rsb{k}") for k in range(8)]
                for k in range(8):
                    nc.sync.dma_start(rsb[k][:], rs_out_h[th][128 * k:128 * (k + 1), :])
                    nc.vector.tensor_tensor(x2_T[k][:], x2_T[k][:], rsb[k][:], ALU.add)

                # rmsnorm over features via ones-matmul
                sq = [ar.tile([128, TH], BF16, tag=f"sq{k}", name=f"sq{k}") for k in range(8)]
                for k in range(8):
                    nc.scalar.activation(sq[k][:], x2_T[k][:], AF.Square)
                pss = pp1.tile([1, TH], F32, tag="pss", name="pss")
                for k in range(8):
                    nc.tensor.matmul(pss[:], ones_t[:], sq[k][:], start=(k == 0), stop=(k == 7))
                rrow = st.tile([1, TH], F32, tag="rrow", name="rrow")
                nc.scalar.activation(rrow[:], pss[:], AF.Sqrt, scale=1.0 / D_MODEL,
                                     bias=eps_t[0:1, 0:1])
                rrec = st.tile([1, TH], F32, tag="rrec", name="rrec")
                nc.vector.reciprocal(rrec[:], rrow[:])
                rbf = st.tile([1, TH], BF16, tag="rbf", name="rbf")
                nc.vector.tensor_copy(rbf[:], rrec[:])
                pr2 = pp1.tile([128, TH], F32, tag="pr2", name="pr2")
                nc.tensor.matmul(pr2[:], onesr_t[:], rbf[:], start=True, stop=True)
                x2n = [ar.tile([128, TH], BF16, tag=f"x2n{k}", name=f"x2n{k}")
                       for k in range(8)]
                for k in range(8):
                    nc.vector.tensor_tensor(x2n[k][:], x2_T[k][:], pr2[:], ALU.mult)

                # c_fc (fp8 DoubleRow) + relu^2 in bf16 — the squared path is
                # too fp8-sensitive for the error gate
                hh_t = [ar.tile([128, TH], BF16, tag=f"hh{k}", name=f"hh{k}")
                        for k in range(16)]
                for m in range(16):
                    pm = ppm.tile([128, TH], F32, tag="pmm", name="pmm")
                    for k in range(8):
                        nc.tensor.matmul(pm[:], w_fc_t[k][:, 128 * m:128 * (m + 1)],
                                         x2n[k][:], start=(k == 0), stop=(k == 7))
                    rl = st.tile([128, TH], BF16, tag="rl", name="rl")
                    nc.scalar.activation(rl[:], pm[:], AF.Relu)
                    nc.vector.tensor_tensor(hh_t[m][:], rl[:], rl[:], ALU.mult)
                # c_proj + residual
                fin = [ar.tile([128, TH], F32, tag=f"fin{k}", name=f"fin{k}") for k in range(8)]
                for m in range(8):
                    pm = ppm.tile([128, TH], F32, tag="pmm", name="pmm")
                    for k in range(16):
                        nc.tensor.matmul(pm[:], w_pr_t[k][:, 128 * m:128 * (m + 1)],
                                         hh_t[k][:], start=(k == 0), stop=(k == 15))
                    nc.vector.tensor_tensor(fin[m][:], x2_T[m][:], pm[:], ALU.add)
                # transpose to token-major + store
                for i in range(TH // 128):
                    for h in range(2):
                        pt = ppt.tile([128, 512], F32, tag="ptx", name="ptx")
                        for q in range(4):
                            m = 4 * h + q
                            nc.tensor.transpose(pt[:, 128 * q:128 * (q + 1)],
                                                fin[m][:, 128 * i:128 * (i + 1)], idf_t[:])
                        ot = st.tile([128, 512], BF16, tag="ot", name="ot")
                        nc.scalar.copy(ot[:], pt[:])
                        nc.sync.dma_start(out[t0 + 128 * i:t0 + 128 * (i + 1),
                                              512 * h:512 * (h + 1)], ot[:])

    nc.compile()
    return nc


def _prep_inputs(inputs):
    x = np.asarray(inputs['x'], np.float32)
    in_proj_w = np.asarray(inputs['in_proj_w'], np.float32)
    conv_w = np.asarray(inputs['conv_w'], np.float32)
    conv_b = np.asarray(inputs['conv_b'], np.float32)
    x_proj_w = np.asarray(inputs['x_proj_w'], np.float32)
    dt_proj_w = np.asarray(inputs['dt_proj_w'], np.float32)
    dt_proj_b = np.asarray(inputs['dt_proj_b'], np.float32)
    A_log = np.asarray(inputs['A_log'], np.float32)
    D = np.asarray(inputs['D'], np.float32)
    out_proj_w = np.asarray(inputs['out_proj_w'], np.float32)
    c_fc_w = np.asarray(inputs['c_fc_w'], np.float32)
    c_proj_w = np.asarray(inputs['c_proj_w'], np.float32)

    import ml_dtypes
    bf = lambda a: np.ascontiguousarray(a).astype(ml_dtypes.bfloat16)
    f32 = lambda a: np.ascontiguousarray(a, np.float32)
    f8 = lambda a: np.ascontiguousarray(a).astype(ml_dtypes.float8_e4m3)

    r8 = np.zeros((8, 128), np.float32)         # r8[i, m] = 1 iff m%8 == i
    g8 = np.zeros((128, 8), np.float32)         # g8[k, i] = 8 iff k%8 == i
    for m in range(128):
        r8[m % 8, m] = 1.0
        g8[m, m % 8] = 8.0
    s01n = np.zeros((D_STATE, 128), np.float32)
    s01p = np.zeros((D_STATE, 128), np.float32)
    for m in range(128):
        s01n[m // 8, m] = -1.0
        s01p[m // 8, m] = 1.0
    ident = np.eye(128, dtype=np.float32)

    def col_fold(a):
        # (EL,) or (EL, w) -> (128, NET*w): cols [w*k:w*(k+1)] = rows of e-tile k
        a = a.reshape(EL, -1)
        w = a.shape[1]
        o = np.zeros((128, NET * w), np.float32)
        for k in range(NET):
            o[:, w * k:w * (k + 1)] = a[128 * k:128 * (k + 1)]
        return o

    in_maps = []
    for c in range(8):
        b, r = c // 4, c % 4
        sl = slice(EL * r, EL * (r + 1))
        negA_ = np.zeros((128, NJ), np.float32)
        p = np.arange(128)
        for j in range(NJ):
            e = EL * r + 8 * j + (p % 8)
            s = p // 8
            negA_[:, j] = np.exp(A_log[e, s])
        import ml_dtypes
        # one packed bf16-typed buffer; fp8/f32 live as bitcast byte sections
        pball = np.zeros((7656, 1024), ml_dtypes.bfloat16)

        def put(rows, a):
            a = np.ascontiguousarray(a).astype(ml_dtypes.bfloat16).reshape(-1)
            n = a.size // 1024
            if n * 1024 != a.size:
                pball[rows, 0:a.size] = a
            else:
                pball[rows:rows + n] = a.reshape(n, 1024)
        put(0, x[b])                              # xb 0:2048
        put(2048, x[b][TQ * r:TQ * (r + 1)])      # xq 2048:2560
        put(2560, c_fc_w.T)                       # w_fc full 2560:4608
        put(4608, c_proj_w.T)                     # w_pr full 4608:6656
        put(6656, x_proj_w[:, sl].T)              # w_xp_T 6656:6704
        put(6704, dt_proj_w[sl].T)                # w_dt_T 6704:6736
        put(6736, ident)                          # ident_bf 6736:6752
        put(6752, s01n)                           # 6752:6754
        put(6754, s01p)                           # 6754:6756
        put(6756, r8)                             # 6756:6757
        put(6757, g8)                             # 6757:6758
        put(6758, np.ones(128, np.float32))       # ones row
        # fp8 section: rows 6760:7528 = [768, 2048] fp8 bytes
        pf8 = np.zeros((768, 2048), ml_dtypes.float8_e4m3)
        pf8[0:512] = f8(np.concatenate([in_proj_w[sl], in_proj_w[D_INNER:][sl]], 0).T
                        * 256.0).reshape(512, 2048)
        pf8[512:768] = f8(out_proj_w[:, sl].T * 256.0).reshape(256, 2048)
        pball[6760:7528] = pf8.view(np.uint8).view(ml_dtypes.bfloat16)
        # f32 section: rows 4456:4584 = [128, 512] f32 (col-sliced consts)
        pf32 = np.zeros((128, 512), np.float32)
        pf32[:, 0:16] = col_fold(conv_w[sl])
        pf32[:, 16:20] = col_fold(conv_b[sl])
        pf32[:, 20:24] = col_fold(-dt_proj_b[sl])
        pf32[:, 24:88] = negA_
        pf32[:, 88:92] = col_fold(D[sl]) * 8.0
        pf32[:, 92:220] = ident
        pball[7528:7656] = pf32.view(np.uint8).view(ml_dtypes.bfloat16)
        in_maps.append({'pball': pball})
    return in_maps


def kernel(**inputs) -> np.ndarray:
    if 'nc' not in _CACHE:
        _CACHE['nc'] = _build()
    nc = _CACHE['nc']
    in_maps = _prep_inputs(inputs)
    res = run_bass_kernel_spmd(nc, in_maps, core_ids=list(range(8)))
    out = np.zeros((B, T, D_MODEL), np.float32)
    for c in range(8):
        b, r = c // 4, c % 4
        out[b, TQ * r:TQ * (r + 1), :] = np.asarray(res.results[c]['out'],
                                                      dtype=np.float32)
    return out

